# revision 1
# baseline (speedup 1.0000x reference)
"""Trainium2 Bass kernel for nn_CaptureCityHead (2-layer transformer + score head).

Strategy (8 NeuronCores, sequence-parallel):
  - Host gathers x = node_emb[tile_ids], precomputes RoPE cos/sin tables,
    shards rows S=2048 -> 8 x 256, ships activations TRANSPOSED [D, 256].
  - Each core computes q/k/v for its shard; k^T and v (65-col head-packed,
    with a ones column for the softmax denominator) are all-gathered across
    the 8 cores in bf16. AGs are split into head-halves (A: heads 0-7,
    B: heads 8-15) so attention on group A overlaps the group-B gathers.
  - Activations stay transposed [D(part), m(free)]; weights are used
    as-stored as matmul stationary operands. Cross-partition reductions
    (LayerNorm stats, softmax denominator broadcast) run on the PE via
    ones-vector matmuls.
  - Matmul operands bf16 (fp32 PSUM accumulation); residual stream fp32.
"""
import contextlib

import numpy as np
import ml_dtypes

import concourse.bass as bass
import concourse.mybir as mybir
import concourse.tile as tile
from concourse import bacc
from concourse.bass_utils import run_bass_kernel_spmd

BF = mybir.dt.bfloat16
F32 = mybir.dt.float32
AF = mybir.ActivationFunctionType
OP = mybir.AluOpType

NC_ = 8          # cores
S, D, H, HD, L, MH = 2048, 1024, 16, 64, 2, 1024
SL = S // NC_    # 256 rows per core
PT = D // 128    # 8 partition tiles
LN_EPS = 1e-5
NPBF = ml_dtypes.bfloat16


def build():
    nc = bacc.Bacc("TRN2", target_bir_lowering=False, debug=False, num_devices=NC_)

    # ---------------- DRAM I/O ----------------
    xT_d = nc.dram_tensor("xT", [D, SL], F32, kind="ExternalInput")
    rp_d = nc.dram_tensor("ropeT", [D, SL], F32, kind="ExternalInput")
    bp_d = nc.dram_tensor("biasp", [128, 177], F32, kind="ExternalInput")
    wq_d = nc.dram_tensor("wq", [L, D, D], BF, kind="ExternalInput")
    wk_d = nc.dram_tensor("wk", [L, D, D], BF, kind="ExternalInput")
    wv_d = nc.dram_tensor("wv", [L, D, D], BF, kind="ExternalInput")
    wo_d = nc.dram_tensor("wo", [L, D, D], BF, kind="ExternalInput")
    w1_d = nc.dram_tensor("w1", [L, D, 2 * D], BF, kind="ExternalInput")
    w2_d = nc.dram_tensor("w2", [L, 2 * D, D], BF, kind="ExternalInput")
    ws0_d = nc.dram_tensor("ws0", [D, MH], BF, kind="ExternalInput")
    ws1_d = nc.dram_tensor("ws1", [MH, MH], BF, kind="ExternalInput")
    ws2_d = nc.dram_tensor("ws2", [MH, 1], BF, kind="ExternalInput")
    out_d = nc.dram_tensor("logits", [1, SL], F32, kind="ExternalOutput")

    with tile.TileContext(nc) as tc:
        ctx = contextlib.ExitStack()
        with ctx:
            dramp = ctx.enter_context(tc.tile_pool(name="dramp", bufs=1, space="DRAM"))
            rg = [list(range(NC_))]

            def dtile(shape, tag, shared=False):
                return dramp.tile(shape, BF, tag=tag, name=tag,
                                  addr_space="Shared" if shared else "Local")

            dum_in = dtile([1, 16], "dum_in")
            dum_out = dtile([NC_, 16], "dum_out", shared=True)
            # per layer, per half (A=heads 0-7, B=8-15)
            kag_in = [[dtile([D // 2, SL], f"kag_in{l}{g}") for g in range(2)]
                      for l in range(L)]
            kag_out = [[dtile([NC_ * D // 2, SL], f"kag_out{l}{g}", shared=True)
                        for g in range(2)] for l in range(L)]
            vag_in = [[dtile([SL, 8 * 65], f"vag_in{l}{g}") for g in range(2)]
                      for l in range(L)]
            vag_out = [[dtile([S, 8 * 65], f"vag_out{l}{g}", shared=True)
                        for g in range(2)] for l in range(L)]

            persist = ctx.enter_context(tc.tile_pool(name="persist", bufs=1))
            sb = ctx.enter_context(tc.tile_pool(name="sb", bufs=1))
            psum = ctx.enter_context(tc.tile_pool(name="psum", bufs=2, space="PSUM"))

            def sbt(shape, dtype, tag, name, bufs):
                return sb.tile(shape, dtype, tag=tag, name=name, bufs=bufs)

            # ---------------- persistent/setup ----------------
            ones_sq = persist.tile([128, 128], F32, tag="ones_sq", name="ones_sq")
            nc.vector.memset(ones_sq[:], 1.0)
            ones_col = ones_sq[:, 0:1]
            ones_row = ones_sq[0:1, :]
            eps_t = persist.tile([1, 1], F32, tag="eps_t", name="eps_t")
            nc.vector.memset(eps_t[:], LN_EPS)

            xt_all = persist.tile([128, PT, SL], F32, tag="xt_all", name="xt_all")
            nc.sync.dma_start(out=xt_all[:],
                              in_=xT_d.ap().rearrange("(t p) m -> p t m", p=128))
            rp_all = persist.tile([128, PT, SL], F32, tag="rp_all", name="rp_all")
            nc.sync.dma_start(out=rp_all[:],
                              in_=rp_d.ap().rearrange("(t p) m -> p t m", p=128))

            def xt(i):
                return xt_all[:, i, :]

            def rp(i):
                return rp_all[:, i, :]

            biasp = persist.tile([128, 177], F32, tag="biasp", name="biasp")
            nc.sync.dma_start(out=biasp[:], in_=bp_d.ap())
            # col layout: bq 0-15 (l*8+o), bk 16, boe 32, b2 48, g1 64, be1 80,
            # g2 96, be2 112, b1 128-159 (l*16+ho), bs0 160, bs1 168, bs2 @ [0,176]
            bcol = {
                "bq": 0, "bk": 16, "boe": 32, "b2": 48,
                "g1": 64, "be1": 80, "g2": 96, "be2": 112,
            }

            def bc(nm, l, o):
                c = bcol[nm] + l * 8 + o
                return biasp[:, c:c + 1]

            def b1c(l, ho):
                c = 128 + l * 16 + ho
                return biasp[:, c:c + 1]

            ws2_sb = persist.tile([128, PT], BF, tag="ws2_sb", name="ws2_sb")
            nc.sync.dma_start(out=ws2_sb[:],
                              in_=ws2_d.ap().rearrange("(c p) o -> p (c o)", p=128))

            def load_wbig(src_ap, nm):
                """One [1024, 1024] weight block -> single [128, 8, 1024] tile."""
                t = sbt([128, PT, 1024], BF, "wbig", f"w_{nm}", 2)
                nc.sync.dma_start(out=t[:],
                                  in_=src_ap.rearrange("(t p) n -> p t n", p=128))
                return t

            def proj_T(w_all, rhs_fn, bias_fn, out_tag, relu=False, bufs=9,
                       o_range=range(PT)):
                """yT[o] = act(sum_p w[:, p, o*128]^T @ rhs(p) + bias(o)) -> bf16."""
                outs = []
                for o in o_range:
                    ps = psum.tile([128, SL], F32, tag="pmm", bufs=1, name=f"ps_{out_tag}{o}")
                    for p in range(PT):
                        nc.tensor.matmul(
                            ps[:], w_all[:, p, o * 128:(o + 1) * 128], rhs_fn(p),
                            start=(p == 0), stop=(p == PT - 1))
                    t = sbt([128, SL], BF, out_tag, f"{out_tag}{o}", bufs)
                    if relu:
                        nc.scalar.activation(t[:], ps[:], AF.Relu, bias=bias_fn(o),
                                             scale=1.0)
                    else:
                        nc.vector.tensor_scalar_add(t[:], ps[:], bias_fn(o))
                    outs.append(t)
                return outs

            # ================= layers =================
            for l in range(L):
                # --- rope: xr (bf16) ---
                rope_map = [
                    (0, 0, 2, 2, -1), (1, 1, 3, 3, -1),
                    (0, 2, 2, 0, +1), (1, 3, 3, 1, +1),
                    (4, 4, 6, 6, -1), (5, 5, 7, 7, -1),
                    (4, 6, 6, 4, +1), (5, 7, 7, 5, +1),
                ]
                xr = []
                for i in range(PT):
                    xa, ra, xb_, rb, sign = rope_map[i]
                    t1 = sbt([128, SL], F32, "ropet1", f"rt1_{l}_{i}", 3)
                    t2 = sbt([128, SL], F32, "ropet2", f"rt2_{l}_{i}", 3)
                    nc.vector.tensor_mul(t1[:], xt(xa), rp(ra))
                    nc.vector.tensor_mul(t2[:], xt(xb_), rp(rb))
                    xo = sbt([128, SL], BF, "xr", f"xr{l}_{i}", 9)
                    if sign > 0:
                        nc.vector.tensor_add(xo[:], t1[:], t2[:])
                    else:
                        nc.vector.tensor_sub(xo[:], t1[:], t2[:])
                    xr.append(xo)

                # --- k projection halves + AGs ---
                wk_t = load_wbig(wk_d.ap()[l], f"wk{l}")
                kt_a = proj_T(wk_t, lambda p: xr[p][:], lambda o: bc("bk", l, o),
                              "kt", bufs=4, o_range=range(0, 4))
                for o in range(4):
                    nc.sync.dma_start(out=kag_in[l][0][o * 128:(o + 1) * 128, :],
                                      in_=kt_a[o][:])
                nc.gpsimd.collective_compute(
                    "AllGather", OP.bypass, replica_groups=rg,
                    ins=[kag_in[l][0][:]], outs=[kag_out[l][0][:]])

                # --- v projection (natural layout, 65-col head packing) ---
                wv_t = load_wbig(wv_d.ap()[l], f"wv{l}")
                xb = []
                for i in range(PT):
                    t = sbt([128, SL], BF, "xb", f"xb{l}_{i}", 9)
                    nc.vector.tensor_copy(t[:], xt(i))
                    xb.append(t)
                va_loc = {}
                for nch in range(2):          # head group (A/B)
                    for mt in range(2):       # row half
                        vt = sbt([128, 8 * 65], BF, "va_loc", f"va_loc{l}_{nch}{mt}", 4)
                        nc.vector.memset(
                            vt[:].rearrange("p (h k) -> p h k", k=65)[:, :, 64:65], 1.0)
                        va_loc[(nch, mt)] = vt
                for mt in range(2):
                    for nch in range(2):
                        ps = psum.tile([128, 512], F32, tag="pmm", bufs=1, name=f"psv{l}{mt}{nch}")
                        for p in range(PT):
                            nc.tensor.matmul(
                                ps[:],
                                xb[p][:, mt * 128:(mt + 1) * 128],
                                wv_t[:, p, nch * 512:(nch + 1) * 512],
                                start=(p == 0), stop=(p == PT - 1))
                        vt = va_loc[(nch, mt)]
                        dst = vt[:].rearrange("p (h k) -> p h k", k=65)[:, :, 0:64]
                        src = ps[:].rearrange("p (h k) -> p h k", k=64)
                        nc.scalar.activation(dst, src, AF.Copy, scale=1.0)
                for nch in range(2):
                    for mt in range(2):
                        nc.sync.dma_start(
                            out=vag_in[l][nch][mt * 128:(mt + 1) * 128, :],
                            in_=va_loc[(nch, mt)][:])
                nc.gpsimd.collective_compute(
                    "AllGather", OP.bypass, replica_groups=rg,
                    ins=[vag_in[l][0][:]], outs=[vag_out[l][0][:]])

                # --- k second half + AG-B pair ---
                kt_b = proj_T(wk_t, lambda p: xr[p][:], lambda o: bc("bk", l, o),
                              "kt", bufs=4, o_range=range(4, 8))
                for o in range(4, 8):
                    nc.sync.dma_start(out=kag_in[l][1][(o - 4) * 128:(o - 3) * 128, :],
                                      in_=kt_b[o - 4][:])
                nc.gpsimd.collective_compute(
                    "AllGather", OP.bypass, replica_groups=rg,
                    ins=[kag_in[l][1][:]], outs=[kag_out[l][1][:]])
                nc.gpsimd.collective_compute(
                    "AllGather", OP.bypass, replica_groups=rg,
                    ins=[vag_in[l][1][:]], outs=[vag_out[l][1][:]])

                # --- q projection ---
                wq_t = load_wbig(wq_d.ap()[l], f"wq{l}")
                qt = proj_T(wq_t, lambda p: xr[p][:], lambda o: bc("bq", l, o),
                            "qt", bufs=9)

                # --- attention, group A then B ---
                ctxT = [sbt([128, SL], BF, "ctxT", f"ctxT{l}_{i}", 9)
                        for i in range(PT)]
                for grp in range(2):
                    # V tiles for this head group: [128, 8, 520] x2 (j 0-7, 8-15)
                    va_t = []
                    vv = vag_out[l][grp][:].rearrange("(j p) n -> p j n", p=128)
                    for jg in range(2):
                        t = sbt([128, 8, 8 * 65], BF, "va", f"va{l}_{grp}{jg}", 4)
                        nc.sync.dma_start(out=t[:], in_=vv[:, jg * 8:(jg + 1) * 8, :])
                        va_t.append(t)
                    kk = kag_out[l][grp][:].rearrange(
                        "(c t p) m -> p t c m", c=NC_, t=4, p=128)
                    for pt_i in range(4 * grp, 4 * grp + 4):
                        ktp_t = sbt([128, NC_, SL], BF, "ktp", f"ktp{l}_{pt_i}", 3)
                        nc.sync.dma_start(out=ktp_t[:], in_=kk[:, pt_i - 4 * grp, :, :])
                        # both halves interleaved: their K=64 score matmuls sit in
                        # different PE row groups (0-63 vs 64-127) and overlap.
                        h0, h1 = 2 * pt_i, 2 * pt_i + 1
                        hh0, hh1 = h0 - 8 * grp, h1 - 8 * grp
                        q0 = qt[pt_i][0:64, :]
                        q1 = qt[pt_i][64:128, :]
                        cps = {
                            0: psum.tile([65, SL], F32, tag="pctx", name=f"cps{l}_{h0}"),
                            1: psum.tile([65, SL], F32, tag="pctx", name=f"cps{l}_{h1}"),
                        }
                        for g4 in range(8):          # 2 j-chunks per exp op
                            js = [2 * g4 + i for i in range(2)]
                            ex = {}
                            for half, qh in ((0, q0), (1, q1)):
                                sc4 = psum.tile([128, 2 * SL], F32, tag="psc", bufs=4,
                                                name=f"sc{l}_{2*pt_i+half}_{g4}")
                                for i, j in enumerate(js):
                                    nc.tensor.matmul(
                                        sc4[:, i * SL:(i + 1) * SL],
                                        ktp_t[half * 64:(half + 1) * 64, j // 2,
                                              (j % 2) * 128:(j % 2 + 1) * 128],
                                        qh, start=True, stop=True)
                                e = sbt([128, 2 * SL], BF, "exp",
                                        f"ex{l}_{2*pt_i+half}_{g4}", 4)
                                nc.scalar.activation(e[:], sc4[:], AF.Exp, scale=0.125)
                                ex[half] = e
                            for half, hh in ((0, hh0), (1, hh1)):
                                for i, j in enumerate(js):
                                    nc.tensor.matmul(
                                        cps[half][:],
                                        va_t[j // 8][:, j % 8, hh * 65:(hh + 1) * 65],
                                        ex[half][:, i * SL:(i + 1) * SL],
                                        start=(j == 0), stop=(j == 15))
                        for half, hh in ((0, hh0), (1, hh1)):
                            h = 2 * pt_i + half
                            ctx_ps = cps[half]
                            ctx_sb = sbt([65, SL], F32, "ctxs", f"ctxs{l}_{h}", 3)
                            nc.vector.tensor_copy(ctx_sb[:], ctx_ps[:])
                            denr = sbt([65, SL], F32, "denr", f"denr{l}_{h}", 3)
                            nc.vector.reciprocal(denr[64:65, :], ctx_sb[64:65, :])
                            bc_ps = psum.tile([64, SL], F32, tag="pbc", bufs=1,
                                              name=f"bc{l}_{h}")
                            nc.tensor.matmul(bc_ps[:], ones_sq[64:65, 0:64],
                                             denr[64:65, :], start=True, stop=True)
                            cn = sbt([64, SL], BF, "cn", f"cn{l}_{h}", 3)
                            nc.vector.tensor_mul(cn[:], ctx_sb[0:64, :], bc_ps[:])
                            nc.sync.dma_start(
                                out=ctxT[pt_i][half * 64:(half + 1) * 64, :], in_=cn[:])

                # --- out projection + residual into xt ---
                wo_t = load_wbig(wo_d.ap()[l], f"wo{l}")
                for o in range(PT):
                    ps = psum.tile([128, SL], F32, tag="pmm", bufs=1, name=f"pso{l}_{o}")
                    for p in range(PT):
                        nc.tensor.matmul(
                            ps[:], wo_t[:, p, o * 128:(o + 1) * 128], ctxT[p][:],
                            start=(p == 0), stop=(p == PT - 1))
                    tmp = sbt([128, SL], F32, "evac", f"evo{l}_{o}", 3)
                    nc.vector.tensor_scalar_add(tmp[:], ps[:], bc("boe", l, o))
                    nc.vector.tensor_add(xt(o), tmp[:], xt(o))

                # --- LN (in place on xt) ---
                def layer_norm(g_nm, be_nm):
                    mean_ps = psum.tile([1, SL], F32, tag="pctx", name="mean_ps")
                    for o in range(PT):
                        nc.tensor.matmul(mean_ps[:], ones_col, xt(o),
                                         start=(o == 0), stop=(o == PT - 1))
                    sq_ps = psum.tile([1, SL], F32, tag="pctx", name="sq_ps")
                    for o in range(PT):
                        sq = sbt([128, SL], F32, "sq", f"sq{o}", 2)
                        nc.vector.tensor_mul(sq[:], xt(o), xt(o))
                        nc.tensor.matmul(sq_ps[:], ones_col, sq[:],
                                         start=(o == 0), stop=(o == PT - 1))
                    mean = sbt([1, SL], F32, "lnm", "mean", 2)
                    nc.scalar.mul(mean[:], mean_ps[:], 1.0 / D)
                    m2 = sbt([1, SL], F32, "lnm2", "m2", 2)
                    nc.vector.tensor_mul(m2[:], mean[:], mean[:])
                    var = sbt([1, SL], F32, "lnv", "var", 2)
                    nc.scalar.mul(var[:], sq_ps[:], 1.0 / D)
                    var2 = sbt([1, SL], F32, "lnv2", "var2", 2)
                    nc.vector.tensor_sub(var2[:], var[:], m2[:])
                    # std = sqrt(var + eps); rstd = 1/std
                    std = sbt([1, SL], F32, "lnstd", "std", 2)
                    nc.scalar.activation(std[:], var2[:], AF.Sqrt, bias=eps_t[0:1, 0:1],
                                         scale=1.0)
                    rstd = sbt([1, SL], F32, "lnr", "rstd", 2)
                    nc.vector.reciprocal(rstd[:], std[:])
                    prem = sbt([1, SL], F32, "lnp", "prem", 2)
                    nc.vector.tensor_mul(prem[:], mean[:], rstd[:])
                    # one PSUM bank holds both broadcasts side by side
                    lnb = psum.tile([128, 2 * SL], F32, tag="pbc", bufs=1, name="lnb")
                    nc.tensor.matmul(lnb[:, 0:SL], ones_row, rstd[:],
                                     start=True, stop=True)
                    nc.tensor.matmul(lnb[:, SL:2 * SL], ones_row, prem[:],
                                     start=True, stop=True)
                    for o in range(PT):
                        t1 = sbt([128, SL], F32, "lnt1", f"lnt1_{o}", 3)
                        nc.vector.tensor_mul(t1[:], xt(o), lnb[:, 0:SL])
                        t2 = sbt([128, SL], F32, "lnt2", f"lnt2_{o}", 3)
                        nc.vector.tensor_sub(t2[:], t1[:], lnb[:, SL:2 * SL])
                        nc.vector.tensor_scalar(
                            out=xt(o), in0=t2[:], scalar1=bc(g_nm, l, o),
                            scalar2=bc(be_nm, l, o), op0=OP.mult, op1=OP.add)

                layer_norm("g1", "be1")
                x1b = []
                for o in range(PT):
                    t = sbt([128, SL], BF, "x1b", f"x1b{l}_{o}", 9)
                    nc.vector.tensor_copy(t[:], xt(o))
                    x1b.append(t)

                # --- FFN ---
                w1a_t = load_wbig(w1_d.ap()[l][:, 0:D], f"w1a{l}")
                w1b_t = load_wbig(w1_d.ap()[l][:, D:2 * D], f"w1b{l}")
                h1b = []
                for ho in range(16):
                    wt_t = w1a_t if ho < 8 else w1b_t
                    oo = ho % 8
                    ps = psum.tile([128, SL], F32, tag="pmm", bufs=1, name=f"psf{l}_{ho}")
                    for p in range(PT):
                        nc.tensor.matmul(
                            ps[:], wt_t[:, p, oo * 128:(oo + 1) * 128], x1b[p][:],
                            start=(p == 0), stop=(p == PT - 1))
                    hb = sbt([128, SL], BF, "h1b", f"h1b{l}_{ho}", 17)
                    nc.scalar.activation(hb[:], ps[:], AF.Relu, bias=b1c(l, ho),
                                         scale=1.0)
                    h1b.append(hb)
                w2a_t = load_wbig(w2_d.ap()[l][0:D, :], f"w2a{l}")
                w2b_t = load_wbig(w2_d.ap()[l][D:2 * D, :], f"w2b{l}")
                for o in range(PT):
                    ps = psum.tile([128, SL], F32, tag="pmm", bufs=1, name=f"psf2{l}_{o}")
                    for hc in range(16):
                        w_all = w2a_t if hc < 8 else w2b_t
                        nc.tensor.matmul(
                            ps[:], w_all[:, hc % 8, o * 128:(o + 1) * 128], h1b[hc][:],
                            start=(hc == 0), stop=(hc == 15))
                    tmp = sbt([128, SL], F32, "evac", f"evf{l}_{o}", 3)
                    nc.vector.tensor_scalar_add(tmp[:], ps[:], bc("b2", l, o))
                    nc.vector.tensor_add(xt(o), tmp[:], xt(o))

                layer_norm("g2", "be2")

            # ================= head MLP =================
            xb2 = []
            for i in range(PT):
                t = sbt([128, SL], BF, "xb2", f"xb2_{i}", 9)
                nc.vector.tensor_copy(t[:], xt(i))
                xb2.append(t)
            ws0_t = load_wbig(ws0_d.ap(), "ws0")
            h0 = proj_T(ws0_t, lambda p: xb2[p][:],
                        lambda o: biasp[:, 160 + o:160 + o + 1], "h0", relu=True)
            ws1_t = load_wbig(ws1_d.ap(), "ws1")
            h1 = proj_T(ws1_t, lambda p: h0[p][:],
                        lambda o: biasp[:, 168 + o:168 + o + 1], "h1", relu=True)
            lg_ps = psum.tile([1, SL], F32, tag="pctx", name="lg_ps")
            for p in range(PT):
                nc.tensor.matmul(lg_ps[:], ws2_sb[:, p:p + 1], h1[p][:],
                                 start=(p == 0), stop=(p == PT - 1))
            lg = sbt([1, SL], F32, "lg", "lg", 2)
            nc.vector.tensor_scalar_add(lg[:], lg_ps[:], biasp[0:1, 176:177])
            nc.sync.dma_start(out=out_d.ap(), in_=lg[:])

    nc.compile()
    return nc


# ---------------- host side ----------------
_BUILT = {}


def _get_built():
    if "nc" not in _BUILT:
        _BUILT["nc"] = build()
    return _BUILT["nc"]


def _host_prep(inputs):
    inp = {k: (np.asarray(v) if not np.isscalar(v) else v) for k, v in inputs.items()}
    tile_ids = np.asarray(inp["tile_ids"]).astype(np.int64)
    Ny = int(np.asarray(inp["Ny"]))
    node_emb = np.asarray(inp["node_emb"], dtype=np.float32)
    x0 = node_emb[tile_ids]                       # [S, D]

    hh = 256
    theta = (1.0 / (10000.0 ** (np.arange(hh, dtype=np.float32) / hh))).astype(np.float32)
    rows = (tile_ids // Ny).astype(np.float32)
    cols = (tile_ids % Ny).astype(np.float32)
    cr, sr = np.cos(rows[:, None] * theta[None, :]), np.sin(rows[:, None] * theta[None, :])
    cc, sc = np.cos(cols[:, None] * theta[None, :]), np.sin(cols[:, None] * theta[None, :])

    def bf(x):
        return np.ascontiguousarray(np.asarray(x, dtype=np.float32)).astype(NPBF)

    def f32(x):
        return np.ascontiguousarray(np.asarray(x, dtype=np.float32))

    # bias pack [128, 177]
    bp = np.zeros((128, 177), np.float32)

    def pack2(dst_col, arr, n):   # arr [L, n*128] -> cols dst_col + l*8(or16)+o
        a = f32(arr).reshape(L, n, 128).transpose(2, 0, 1).reshape(128, L * n)
        bp[:, dst_col:dst_col + L * n] = a

    bv = f32(inp["bv"])
    wo = f32(inp["wo"])
    bo = f32(inp["bo"])
    boe = np.stack([bv[l] @ wo[l] + bo[l] for l in range(L)]).astype(np.float32)
    pack2(0, inp["bq"], 8)
    pack2(16, inp["bk"], 8)
    pack2(32, boe, 8)
    pack2(48, inp["b2"], 8)
    pack2(64, inp["g1"], 8)
    pack2(80, inp["be1"], 8)
    pack2(96, inp["g2"], 8)
    pack2(112, inp["be2"], 8)
    pack2(128, inp["b1"], 16)
    bp[:, 160:168] = f32(inp["bs0"]).reshape(8, 128).T
    bp[:, 168:176] = f32(inp["bs1"]).reshape(8, 128).T
    bp[0, 176] = float(np.asarray(inp["bs2"]).reshape(-1)[0])

    shared = {
        "biasp": bp,
        "wq": bf(inp["wq"]), "wk": bf(inp["wk"]), "wv": bf(inp["wv"]),
        "wo": bf(inp["wo"]), "w1": bf(inp["w1"]), "w2": bf(inp["w2"]),
        "ws0": bf(inp["ws0"]), "ws1": bf(inp["ws1"]), "ws2": bf(inp["ws2"]),
    }

    in_maps = []
    for c in range(NC_):
        sl = slice(c * SL, (c + 1) * SL)
        m = dict(shared)
        m["xT"] = np.ascontiguousarray(x0[sl].T).astype(np.float32)
        m["ropeT"] = np.ascontiguousarray(
            np.concatenate([cr[sl].T, sr[sl].T, cc[sl].T, sc[sl].T], axis=0)
        ).astype(np.float32)
        in_maps.append(m)
    return in_maps


def kernel(**inputs):
    nc = _get_built()
    in_maps = _host_prep(inputs)
    res = run_bass_kernel_spmd(nc, in_maps, core_ids=list(range(NC_)))
    logits = np.concatenate(
        [np.asarray(res.results[c]["logits"]).reshape(SL) for c in range(NC_)])
    return logits.astype(np.float32)


if __name__ == "__main__":
    data = np.load("/root/problem/ref_data.npz")
    expected = data["__expected"]
    inputs = {k: data[k] for k in data.files if k != "__expected"}
    got = kernel(**inputs)
    err = np.abs(got - expected)
    rel = np.linalg.norm(got - expected) / np.linalg.norm(expected)
    print("max abs err:", err.max(), "rel l2:", rel)



# revision 22
# speedup vs baseline: 1.0990x; 1.0990x over previous
"""Trainium2 Bass kernel for nn_CaptureCityHead (2-layer transformer + score head).

v2: fp8(e4m3) DoubleRow matmuls for qkv/o projections, scores and attn*V
(2x PE throughput); fp8 collectives (half payload); softmax 1/den and LN
rsqrt via Ln/Exp on the ACT engine (single act table, no DVE reciprocal);
rope/casts/residual adds moved to the idle Pool (gpsimd) engine.

Sharding: sequence-parallel S=2048 -> 8 x 256 rows, transposed activations
[D(part), m(free)]; K^T and V (65-col head-packed with a ones column for
the softmax denominator) all-gathered per head-half in fp8. FFN and score
head stay bf16 (fp8 noise there would exceed the error budget).
"""
import contextlib
import math

import numpy as np
import ml_dtypes

import concourse.bass as bass
import concourse.mybir as mybir
import concourse.tile as tile
from concourse import bacc
from concourse.bass_utils import run_bass_kernel_spmd

BF = mybir.dt.bfloat16
F32 = mybir.dt.float32
F8 = mybir.dt.float8e4
AF = mybir.ActivationFunctionType
OP = mybir.AluOpType
DR = mybir.MatmulPerfMode.DoubleRow

NC_ = 8          # cores
S, D, H, HD, L, MH = 2048, 1024, 16, 64, 2, 1024
SL = S // NC_    # 256 rows per core
PT = D // 128    # 8 partition tiles
LN_EPS = 1e-5
NPBF = ml_dtypes.bfloat16
NPF8 = ml_dtypes.float8_e4m3

WS = 128.0       # fp8 weight scale (wq/wk/wv/wo * 128)
IWS = 1.0 / WS
CTXS = 64.0      # ctx normalize scale (cn = ctx * 64/den)
LN64 = math.log(CTXS)


def build():
    nc = bacc.Bacc("TRN2", target_bir_lowering=False, debug=False, num_devices=NC_)

    # ---------------- DRAM I/O ----------------
    xT_d = nc.dram_tensor("xT", [D, SL], F32, kind="ExternalInput")
    rp_d = nc.dram_tensor("ropeT", [D, SL], F32, kind="ExternalInput")
    bp_d = nc.dram_tensor("biasp", [128, 177], F32, kind="ExternalInput")
    br_d = nc.dram_tensor("biasr", [2, 32 * 128], F32, kind="ExternalInput")
    # fp8 DoubleRow weights: [128, t2(4)*two(2)*1024]
    wq_d = nc.dram_tensor("wq", [L, 128, 8 * 1024], F8, kind="ExternalInput")
    wk_d = nc.dram_tensor("wk", [L, 128, 8 * 1024], F8, kind="ExternalInput")
    wv_d = nc.dram_tensor("wv", [L, 128, 8 * 1024], F8, kind="ExternalInput")
    wo_d = nc.dram_tensor("wo", [L, 128, 8 * 1024], F8, kind="ExternalInput")
    w1_d = nc.dram_tensor("w1", [L, D, 2 * D], BF, kind="ExternalInput")
    w2_d = nc.dram_tensor("w2", [L, 2 * D, D], BF, kind="ExternalInput")
    ws0_d = nc.dram_tensor("ws0", [D, MH], BF, kind="ExternalInput")
    ws1_d = nc.dram_tensor("ws1", [MH, MH], BF, kind="ExternalInput")
    ws2_d = nc.dram_tensor("ws2", [MH, 1], BF, kind="ExternalInput")
    o2_d = nc.dram_tensor("ones2", [2, 128], BF, kind="ExternalInput")
    out_d = nc.dram_tensor("logits", [1, SL], F32, kind="ExternalOutput")

    with tile.TileContext(nc) as tc:
        ctx = contextlib.ExitStack()
        with ctx:
            dramp = ctx.enter_context(tc.tile_pool(name="dramp", bufs=1, space="DRAM"))
            rg = [list(range(NC_))]

            def dtile(shape, tag, shared=False):
                return dramp.tile(shape, F8, tag=tag, name=tag,
                                  addr_space="Shared" if shared else "Local")

            # per layer, per half (A=heads 0-7, B=8-15), all fp8
            kag_in = [[dtile([D // 2, SL], f"kag_in{l}{g}") for g in range(2)]
                      for l in range(L)]
            kag_out = [[dtile([NC_ * D // 2, SL], f"kag_out{l}{g}", shared=True)
                        for g in range(2)] for l in range(L)]
            vag_in = [[dtile([SL, 8 * 66], f"vag_in{l}{g}") for g in range(2)]
                      for l in range(L)]
            vag_out = [[dtile([S, 8 * 66], f"vag_out{l}{g}", shared=True)
                        for g in range(2)] for l in range(L)]
            q_scr = dtile([D, SL], "q_scr")   # local scratch for q row shuffle

            persist = ctx.enter_context(tc.tile_pool(name="persist", bufs=1))
            sb = ctx.enter_context(tc.tile_pool(name="sb", bufs=1))
            psum = ctx.enter_context(tc.tile_pool(name="psum", bufs=2, space="PSUM"))

            def sbt(shape, dtype, tag, name, bufs):
                return sb.tile(shape, dtype, tag=tag, name=name, bufs=bufs)

            # ---------------- persistent/setup ----------------
            ones_sq = persist.tile([128, 128], F32, tag="ones_sq", name="ones_sq")
            nc.vector.memset(ones_sq[:], 1.0)
            ones_col = ones_sq[:, 0:1]
            ones_bf = persist.tile([128, 1], BF, tag="ones_bf", name="ones_bf")
            nc.vector.memset(ones_bf[:], 1.0)

            xt_all = persist.tile([128, PT, SL], F32, tag="xt_all", name="xt_all")
            nc.sync.dma_start(out=xt_all[:],
                              in_=xT_d.ap().rearrange("(t p) m -> p t m", p=128))
            rp_all = persist.tile([128, PT, SL], F32, tag="rp_all", name="rp_all")
            nc.sync.dma_start(out=rp_all[:],
                              in_=rp_d.ap().rearrange("(t p) m -> p t m", p=128))

            def xt(i):
                return xt_all[:, i, :]

            def rp(i):
                return rp_all[:, i, :]

            biasp = persist.tile([128, 177], F32, tag="biasp", name="biasp")
            nc.sync.dma_start(out=biasp[:], in_=bp_d.ap())
            # col layout: bq 0-15 (l*8+o), bk 16, boe 32, b2 48,
            # b1 128-159 (l*16+ho), bs0 160, bs1 168, bs2 @ [0,176]
            bcol = {"bq": 0, "bk": 16, "boe": 32, "b2": 48}

            def bc(nm, l, o):
                c = bcol[nm] + l * 8 + o
                return biasp[:, c:c + 1]

            def b1c(l, ho):
                c = 128 + l * 16 + ho
                return biasp[:, c:c + 1]

            # LN stationary rows [2, 32, 128]: row0=g, row1=-be per instance
            biasr = persist.tile([2, 32, 128], F32, tag="biasr", name="biasr")
            nc.sync.dma_start(
                out=biasr[:], in_=br_d.ap().rearrange("r (i n) -> r i n", n=128))

            ws2_sb = persist.tile([128, PT], BF, tag="ws2_sb", name="ws2_sb")
            nc.sync.dma_start(out=ws2_sb[:],
                              in_=ws2_d.ap().rearrange("(c p) o -> p (c o)", p=128))
            # recip broadcast stationary: [2, 128] bf16; row0 -> parts 0-63,
            # row1 -> parts 64-127
            ones2 = persist.tile([2, 128], BF, tag="ones2", name="ones2")
            nc.sync.dma_start(out=ones2[:], in_=o2_d.ap())
            # LN moving helper [2, SL]: row0 = prem (written per LN), row1 = 1
            mvb = persist.tile([2, SL], F32, tag="mvb", name="mvb")
            nc.vector.memset(mvb[:], 1.0)
            ln64_t = persist.tile([8, 1], F32, tag="ln64_t", name="ln64_t")
            nc.vector.memset(ln64_t[:], LN64)
            eps_t = persist.tile([1, 1], F32, tag="eps_t", name="eps_t")
            nc.vector.memset(eps_t[:], LN_EPS)

            def load_w8(src_ap, nm):
                """fp8 DR weight [128, 4, 2, 1024] from dram [128, 8192]."""
                t = sbt([128, 4, 2, 1024], F8, "w8", f"w_{nm}", 3)
                nc.sync.dma_start(
                    out=t[:], in_=src_ap.rearrange("p (t w n) -> p t w n", t=4, w=2))
                return t

            def load_wbig(src_ap, nm):
                """bf16 [1024, 1024] weight -> [128, 8, 1024] tile."""
                t = sbt([128, PT, 1024], BF, "wbig", f"w_{nm}", 2)
                nc.sync.dma_start(out=t[:],
                                  in_=src_ap.rearrange("(t p) n -> p t n", p=128))
                return t

            # ================= layers =================
            for l in range(L):
                # --- rope: xrp fp8 pair tiles [128, 2, SL] x4 ---
                rope_map = [
                    (0, 0, 2, 2, -1), (1, 1, 3, 3, -1),
                    (0, 2, 2, 0, +1), (1, 3, 3, 1, +1),
                    (4, 4, 6, 6, -1), (5, 5, 7, 7, -1),
                    (4, 6, 6, 4, +1), (5, 7, 7, 5, +1),
                ]
                xrp = [sbt([128, 2, SL], F8, "xrp", f"xrp{l}_{j}", 5)
                       for j in range(4)]
                for i in range(PT):
                    xa, ra, xb_, rb, sign = rope_map[i]
                    t1 = sbt([128, SL], F32, "ropet1", f"rt1_{l}_{i}", 2)
                    t2 = sbt([128, SL], F32, "ropet2", f"rt2_{l}_{i}", 2)
                    nc.gpsimd.tensor_mul(t1[:], xt(xa), rp(ra))
                    nc.gpsimd.tensor_mul(t2[:], xt(xb_), rp(rb))
                    dst = xrp[i // 2][:, i % 2, :]
                    if sign > 0:
                        nc.vector.tensor_add(dst, t1[:], t2[:])
                    else:
                        nc.vector.tensor_sub(dst, t1[:], t2[:])
                # x (un-roped) fp8 pairs for V
                xp = [sbt([128, 2, SL], F8, "xp", f"xp{l}_{j}", 5) for j in range(4)]
                for i in range(PT):
                    nc.gpsimd.tensor_copy(xp[i // 2][:, i % 2, :], xt(i))

                # --- K projection (fp8 DR) -> kag halves ---
                wk_t = load_w8(wk_d.ap()[l], f"wk{l}")

                def proj_qk(w_t, bias_nm, o, out_tag, bufs):
                    ps = psum.tile([128, SL], F32, tag="pmm", bufs=1,
                                   name=f"ps_{out_tag}{o}")
                    for t2 in range(4):
                        nc.tensor.matmul(
                            ps[:], w_t[:, t2, :, o * 128:(o + 1) * 128],
                            xrp[t2][:], start=(t2 == 0), stop=(t2 == 3),
                            perf_mode=DR)
                    t = sbt([128, SL], F8, out_tag, f"{out_tag}{l}_{o}", bufs)
                    nc.vector.tensor_scalar(
                        out=t[:], in0=ps[:], scalar1=IWS, scalar2=bc(bias_nm, l, o),
                        op0=OP.mult, op1=OP.add)
                    return t

                for g in range(2):
                    for o in range(4 * g, 4 * g + 4):
                        kt = proj_qk(wk_t, "bk", o, "kt", 4)
                        nc.sync.dma_start(
                            out=kag_in[l][g][(o - 4 * g) * 128:(o - 4 * g + 1) * 128, :],
                            in_=kt[:])
                    if g == 0:
                        nc.gpsimd.collective_compute(
                            "AllGather", OP.bypass, replica_groups=rg,
                            ins=[kag_in[l][0][:]], outs=[kag_out[l][0][:]])

                # --- V projection (fp8 DR, natural layout, 65-col packing) ---
                wv_t = load_w8(wv_d.ap()[l], f"wv{l}")
                va_loc = {}
                for nch in range(2):
                    for mt in range(2):
                        vt = sbt([128, 8 * 66], F8, "va_loc", f"va_loc{l}_{nch}{mt}", 4)
                        nc.vector.memset(
                            vt[:].rearrange("p (h k) -> p h k", k=66)[:, :, 64:65], 1.0)
                        nc.vector.memset(
                            vt[:].rearrange("p (h k) -> p h k", k=66)[:, :, 65:66], 0.0)
                        va_loc[(nch, mt)] = vt
                for nch in range(2):
                    for mt in range(2):
                        ps = psum.tile([128, 512], F32, tag="pmm", bufs=1,
                                       name=f"psv{l}{mt}{nch}")
                        for t2 in range(4):
                            nc.tensor.matmul(
                                ps[:],
                                xp[t2][:, :, mt * 128:(mt + 1) * 128],
                                wv_t[:, t2, :, nch * 512:(nch + 1) * 512],
                                start=(t2 == 0), stop=(t2 == 3), perf_mode=DR)
                        vt = va_loc[(nch, mt)]
                        dst = vt[:].rearrange("p (h k) -> p h k", k=66)[:, :, 0:64]
                        src = ps[:].rearrange("p (h k) -> p h k", k=64)
                        nc.scalar.activation(dst, src, AF.Copy, scale=IWS)
                    for mt in range(2):
                        nc.sync.dma_start(
                            out=vag_in[l][nch][mt * 128:(mt + 1) * 128, :],
                            in_=va_loc[(nch, mt)][:])
                    nc.gpsimd.collective_compute(
                        "AllGather", OP.bypass, replica_groups=rg,
                        ins=[vag_in[l][nch][:]], outs=[vag_out[l][nch][:]])
                # K half B AG after its inputs landed (emitted above), then V-B
                nc.gpsimd.collective_compute(
                    "AllGather", OP.bypass, replica_groups=rg,
                    ins=[kag_in[l][1][:]], outs=[kag_out[l][1][:]])

                # --- Q projection (fp8 DR) -> q_scr roundtrip -> q2 per head ---
                wq_t = load_w8(wq_d.ap()[l], f"wq{l}")
                for o in range(PT):
                    qt = proj_qk(wq_t, "bq", o, "qt", 9)
                    nc.sync.dma_start(out=q_scr[o * 128:(o + 1) * 128, :], in_=qt[:])
                q_view = q_scr[:].rearrange(
                    "(t pp w p) m -> t pp p w m", t=8, pp=2, w=2, p=32)
                q2 = []
                for h in range(H):
                    t = sbt([32, 2, SL], F8, "q2", f"q2_{l}_{h}", 17)
                    nc.sync.dma_start(out=t[:], in_=q_view[h // 2, h % 2])
                    q2.append(t)

                # --- attention, head group A then B ---
                ctxp = [sbt([128, 2, SL], F8, "ctxp", f"ctxp{l}_{j}", 5)
                        for j in range(4)]
                den_all = [None, None]
                ctx_sb = {}
                for grp in range(2):
                    da = sbt([8, SL], F32, "den", f"den{l}_{grp}", 2)
                    den_all[grp] = da
                    # V tiles for this head group: [128, 8, 520] x2 (j 0-7, 8-15)
                    va_t = []
                    vv = vag_out[l][grp][:].rearrange("(j p) n -> p j n", p=128)
                    for jg in range(2):
                        t = sbt([128, 8, 8 * 66], F8, "va", f"va{l}_{grp}{jg}", 4)
                        nc.sync.dma_start(out=t[:], in_=vv[:, jg * 8:(jg + 1) * 8, :])
                        va_t.append(t)
                    # K tiles per head: [32, 2, 8, SL]
                    kk = kag_out[l][grp][:].rearrange(
                        "(c t pp w p) m -> t pp p w c m", c=NC_, t=4, pp=2, w=2, p=32)
                    for hh in range(8):
                        h = grp * 8 + hh
                        ktp = sbt([32, 2, NC_, SL], F8, "ktp", f"ktp{l}_{h}", 2)
                        for w in range(2):
                            nc.sync.dma_start(out=ktp[:, w, :, :],
                                              in_=kk[hh // 2, hh % 2, :, w])
                        cps = psum.tile([66, SL], F32, tag="pctx", bufs=2,
                                        name=f"cps{l}_{h}")
                        for q4 in range(4):      # 4 key-chunks of 256 per psum
                            sc = psum.tile([128, 4 * SL], F32, tag="psc", bufs=2,
                                           name=f"sc{l}_{h}_{q4}")
                            for i in range(4):
                                j = 4 * q4 + i   # key chunk of 128
                                nc.tensor.matmul(
                                    sc[:, i * SL:(i + 1) * SL],
                                    ktp[:, :, j // 2, (j % 2) * 128:(j % 2 + 1) * 128],
                                    q2[h][:], start=True, stop=True, perf_mode=DR)
                            e = sbt([128, 4 * SL], F8, "exp", f"ex{l}_{h}_{q4}", 3)
                            nc.scalar.activation(e[:], sc[:], AF.Exp, scale=0.125)
                            for pi in range(2):  # ctx: 2 DR matmuls (j-pairs)
                                j = 4 * q4 + 2 * pi
                                nc.tensor.matmul(
                                    cps[:],
                                    va_t[j // 8][:, j % 8:j % 8 + 2,
                                                 hh * 66:(hh + 1) * 66],
                                    e[:, 2 * pi * SL:(2 * pi + 2) * SL]
                                    .rearrange("p (w m) -> p w m", w=2),
                                    start=(j == 0), stop=(j == 14), perf_mode=DR)
                        csb = sbt([66, SL], F32, "ctxs", f"ctxs{l}_{h}", 10)
                        nc.vector.tensor_copy(csb[:], cps[:])
                        ctx_sb[h] = csb
                        nc.sync.dma_start(out=da[hh:hh + 1, :], in_=csb[64:65, :])
                    # batch reciprocal via ACT: r = exp(-ln(den) + ln 64)
                    lden = sbt([8, SL], F32, "lden", f"lden{l}_{grp}", 2)
                    nc.scalar.activation(lden[:], da[:], AF.Ln)
                    rden = sbt([8, SL], BF, "rden", f"rden{l}_{grp}", 2)
                    nc.scalar.activation(rden[:], lden[:], AF.Exp, scale=-1.0,
                                         bias=ln64_t[:])
                    for hp in range(4):          # head pairs -> broadcast + cn
                        rdp = sbt([2, SL], BF, "rdp", f"rdp{l}_{grp}_{hp}", 3)
                        nc.sync.dma_start(out=rdp[:], in_=rden[2 * hp:2 * hp + 2, :])
                        bc_ps = psum.tile([128, SL], F32, tag="pbc", bufs=1,
                                          name=f"bc{l}_{grp}_{hp}")
                        nc.tensor.matmul(bc_ps[:], ones2[:], rdp[:],
                                         start=True, stop=True)
                        for half in range(2):
                            h = grp * 8 + 2 * hp + half
                            dst = ctxp[h // 4][(h % 2) * 64:(h % 2) * 64 + 64,
                                               (h // 2) % 2, :]
                            nc.vector.tensor_mul(
                                dst, ctx_sb[h][0:64, :],
                                bc_ps[half * 64:(half + 1) * 64, :])

                # --- out projection (fp8 DR) + residual into xt ---
                wo_t = load_w8(wo_d.ap()[l], f"wo{l}")
                for o in range(PT):
                    ps = psum.tile([128, SL], F32, tag="pmm", bufs=1, name=f"pso{l}_{o}")
                    for t2 in range(4):
                        nc.tensor.matmul(
                            ps[:], wo_t[:, t2, :, o * 128:(o + 1) * 128],
                            ctxp[t2][:], start=(t2 == 0), stop=(t2 == 3),
                            perf_mode=DR)
                    tmp = sbt([128, SL], F32, "evac", f"evo{l}_{o}", 2)
                    nc.vector.tensor_scalar(
                        out=tmp[:], in0=ps[:], scalar1=IWS / CTXS,
                        scalar2=bc("boe", l, o), op0=OP.mult, op1=OP.add)
                    nc.gpsimd.tensor_add(xt(o), tmp[:], xt(o))

                # --- LayerNorm (in place on xt); ln_i 0/1 within layer ---
                def layer_norm(ln_i):
                    mean_ps = psum.tile([1, SL], F32, tag="pctx", bufs=2,
                                        name=f"mean{l}_{ln_i}")
                    for o in range(PT):
                        nc.tensor.matmul(mean_ps[:], ones_col, xt(o),
                                         start=(o == 0), stop=(o == PT - 1))
                    sq_ps = psum.tile([1, SL], F32, tag="pctx", bufs=2,
                                      name=f"sq{l}_{ln_i}")
                    for o in range(PT):
                        sq = sbt([128, SL], BF, "sq", f"sq{o}", 2)
                        nc.vector.tensor_mul(sq[:], xt(o), xt(o))
                        nc.tensor.matmul(sq_ps[:], ones_bf[:],
                                         sq[:], start=(o == 0), stop=(o == PT - 1))
                    mean = sbt([1, SL], F32, "lnm", "mean", 2)
                    nc.scalar.activation(mean[:], mean_ps[:], AF.Copy, scale=1.0 / D)
                    m2 = sbt([1, SL], F32, "lnm2", "m2", 2)
                    nc.vector.tensor_mul(m2[:], mean[:], mean[:])
                    var = sbt([1, SL], F32, "lnv", "var", 2)
                    nc.vector.tensor_scalar(
                        out=var[:], in0=sq_ps[:], scalar1=1.0 / D, scalar2=None,
                        op0=OP.mult)
                    var2 = sbt([1, SL], F32, "lnv2", "var2", 2)
                    nc.vector.tensor_sub(var2[:], var[:], m2[:])
                    # rstd = exp(-0.5 ln(var + eps)) on ACT (same table as Exp)
                    lv = sbt([1, SL], F32, "lnlv", "lv", 2)
                    nc.scalar.activation(lv[:], var2[:], AF.Ln, bias=eps_t[:])
                    rstd = sbt([1, SL], F32, "lnr", "rstd", 2)
                    nc.scalar.activation(rstd[:], lv[:], AF.Exp, scale=-0.5)
                    nc.vector.tensor_mul(mvb[0:1, :], mean[:], rstd[:])
                    # A = g*rstd_bc ; B = g*prem_bc - be_bc  (one psum bank)
                    lnab = psum.tile([128, 2 * SL], F32, tag="pbc", bufs=1,
                                     name=f"lnab{l}_{ln_i}")
                    ri = (l * 2 + ln_i) * 8
                    for o in range(PT):
                        st = biasr[:, ri + o, :]
                        nc.tensor.matmul(lnab[:, 0:SL], st[0:1, :], rstd[:],
                                         start=True, stop=True)
                        nc.tensor.matmul(lnab[:, SL:2 * SL], st, mvb[:],
                                         start=True, stop=True)
                        t1 = sbt([128, SL], F32, "lnt1", f"lnt1_{o}", 2)
                        nc.vector.tensor_mul(t1[:], xt(o), lnab[:, 0:SL])
                        nc.vector.tensor_sub(xt(o), t1[:], lnab[:, SL:2 * SL])

                layer_norm(0)
                x1b = []
                for o in range(PT):
                    t = sbt([128, SL], BF, "x1b", f"x1b{l}_{o}", 9)
                    nc.gpsimd.tensor_copy(t[:], xt(o))
                    x1b.append(t)

                # --- FFN (bf16) ---
                w1a_t = load_wbig(w1_d.ap()[l][:, 0:D], f"w1a{l}")
                w1b_t = load_wbig(w1_d.ap()[l][:, D:2 * D], f"w1b{l}")
                h1b = []
                for ho in range(16):
                    wt_t = w1a_t if ho < 8 else w1b_t
                    oo = ho % 8
                    ps = psum.tile([128, SL], F32, tag="pmm", bufs=1, name=f"psf{l}_{ho}")
                    for p in range(PT):
                        nc.tensor.matmul(
                            ps[:], wt_t[:, p, oo * 128:(oo + 1) * 128], x1b[p][:],
                            start=(p == 0), stop=(p == PT - 1))
                    hb = sbt([128, SL], BF, "h1b", f"h1b{l}_{ho}", 17)
                    nc.scalar.activation(hb[:], ps[:], AF.Relu, bias=b1c(l, ho),
                                         scale=1.0)
                    h1b.append(hb)
                w2a_t = load_wbig(w2_d.ap()[l][0:D, :], f"w2a{l}")
                w2b_t = load_wbig(w2_d.ap()[l][D:2 * D, :], f"w2b{l}")
                for o in range(PT):
                    ps = psum.tile([128, SL], F32, tag="pmm", bufs=1, name=f"psf2{l}_{o}")
                    for hc in range(16):
                        w_all = w2a_t if hc < 8 else w2b_t
                        nc.tensor.matmul(
                            ps[:], w_all[:, hc % 8, o * 128:(o + 1) * 128], h1b[hc][:],
                            start=(hc == 0), stop=(hc == 15))
                    tmp = sbt([128, SL], F32, "evac", f"evf{l}_{o}", 2)
                    nc.vector.tensor_scalar_add(tmp[:], ps[:], bc("b2", l, o))
                    nc.gpsimd.tensor_add(xt(o), tmp[:], xt(o))

                layer_norm(1)

            # ================= head MLP (bf16) =================
            def proj_T(w_all, rhs_fn, bias_fn, out_tag, bufs=9):
                outs = []
                for o in range(PT):
                    ps = psum.tile([128, SL], F32, tag="pmm", bufs=1,
                                   name=f"ps_{out_tag}{o}")
                    for p in range(PT):
                        nc.tensor.matmul(
                            ps[:], w_all[:, p, o * 128:(o + 1) * 128], rhs_fn(p),
                            start=(p == 0), stop=(p == PT - 1))
                    t = sbt([128, SL], BF, out_tag, f"{out_tag}{o}", bufs)
                    nc.scalar.activation(t[:], ps[:], AF.Relu, bias=bias_fn(o),
                                         scale=1.0)
                    outs.append(t)
                return outs

            xb2 = []
            for i in range(PT):
                t = sbt([128, SL], BF, "xb2", f"xb2_{i}", 9)
                nc.gpsimd.tensor_copy(t[:], xt(i))
                xb2.append(t)
            ws0_t = load_wbig(ws0_d.ap(), "ws0")
            h0 = proj_T(ws0_t, lambda p: xb2[p][:],
                        lambda o: biasp[:, 160 + o:160 + o + 1], "h0")
            ws1_t = load_wbig(ws1_d.ap(), "ws1")
            h1 = proj_T(ws1_t, lambda p: h0[p][:],
                        lambda o: biasp[:, 168 + o:168 + o + 1], "h1")
            lg_ps = psum.tile([1, SL], F32, tag="pctx", name="lg_ps")
            for p in range(PT):
                nc.tensor.matmul(lg_ps[:], ws2_sb[:, p:p + 1], h1[p][:],
                                 start=(p == 0), stop=(p == PT - 1))
            lg = sbt([1, SL], F32, "lg", "lg", 2)
            nc.vector.tensor_scalar_add(lg[:], lg_ps[:], biasp[0:1, 176:177])
            nc.sync.dma_start(out=out_d.ap(), in_=lg[:])

    nc.compile()
    return nc


# ---------------- host side ----------------
_BUILT = {}


def _get_built():
    if "nc" not in _BUILT:
        _BUILT["nc"] = build()
    return _BUILT["nc"]


def _host_prep(inputs):
    inp = {k: (np.asarray(v) if not np.isscalar(v) else v) for k, v in inputs.items()}
    tile_ids = np.asarray(inp["tile_ids"]).astype(np.int64)
    Ny = int(np.asarray(inp["Ny"]))
    node_emb = np.asarray(inp["node_emb"], dtype=np.float32)
    x0 = node_emb[tile_ids]                       # [S, D]

    hh = 256
    theta = (1.0 / (10000.0 ** (np.arange(hh, dtype=np.float32) / hh))).astype(np.float32)
    rows = (tile_ids // Ny).astype(np.float32)
    cols = (tile_ids % Ny).astype(np.float32)
    cr, sr = np.cos(rows[:, None] * theta[None, :]), np.sin(rows[:, None] * theta[None, :])
    cc, sc = np.cos(cols[:, None] * theta[None, :]), np.sin(cols[:, None] * theta[None, :])

    def bf(x):
        return np.ascontiguousarray(np.asarray(x, dtype=np.float32)).astype(NPBF)

    def f32(x):
        return np.ascontiguousarray(np.asarray(x, dtype=np.float32))

    def f8dr(w):
        """[L, 1024, 1024] f32 -> [L, 128, 8192] fp8 DR layout (scaled)."""
        a = f32(w) * WS
        a = a.reshape(L, 4, 2, 128, 1024).transpose(0, 3, 1, 2, 4)
        return np.ascontiguousarray(a.reshape(L, 128, 8 * 1024)).astype(NPF8)

    # bias pack [128, 177]
    bp = np.zeros((128, 177), np.float32)

    def pack2(dst_col, arr, n):
        a = f32(arr).reshape(L, n, 128).transpose(2, 0, 1).reshape(128, L * n)
        bp[:, dst_col:dst_col + L * n] = a

    bv = f32(inp["bv"])
    wo = f32(inp["wo"])
    bo = f32(inp["bo"])
    boe = np.stack([bv[l] @ wo[l] + bo[l] for l in range(L)]).astype(np.float32)
    pack2(0, inp["bq"], 8)
    pack2(16, inp["bk"], 8)
    pack2(32, boe, 8)
    pack2(48, inp["b2"], 8)
    pack2(128, inp["b1"], 16)
    bp[:, 160:168] = f32(inp["bs0"]).reshape(8, 128).T
    bp[:, 168:176] = f32(inp["bs1"]).reshape(8, 128).T
    bp[0, 176] = float(np.asarray(inp["bs2"]).reshape(-1)[0])

    # biasr [2, 32*128]: row0 = g, row1 = -be per LN instance per o-tile
    br = np.zeros((2, 32 * 128), np.float32)
    lns = [("g1", "be1"), ("g2", "be2")]
    for l in range(L):
        for ln_i, (gn, bn) in enumerate(lns):
            for o in range(8):
                k = (l * 2 + ln_i) * 8 + o
                br[0, k * 128:(k + 1) * 128] = f32(inp[gn])[l][o * 128:(o + 1) * 128]
                br[1, k * 128:(k + 1) * 128] = -f32(inp[bn])[l][o * 128:(o + 1) * 128]

    shared = {
        "biasp": bp, "biasr": br,
        "wq": f8dr(inp["wq"]), "wk": f8dr(inp["wk"]), "wv": f8dr(inp["wv"]),
        "wo": f8dr(inp["wo"]),
        "w1": bf(inp["w1"]), "w2": bf(inp["w2"]),
        "ws0": bf(inp["ws0"]), "ws1": bf(inp["ws1"]), "ws2": bf(inp["ws2"]),
        "ones2": np.concatenate([
            np.concatenate([np.ones((1, 64), np.float32),
                            np.zeros((1, 64), np.float32)], axis=1),
            np.concatenate([np.zeros((1, 64), np.float32),
                            np.ones((1, 64), np.float32)], axis=1),
        ]).astype(NPBF),
    }

    in_maps = []
    for c in range(NC_):
        sl = slice(c * SL, (c + 1) * SL)
        m = dict(shared)
        m["xT"] = np.ascontiguousarray(x0[sl].T).astype(np.float32)
        m["ropeT"] = np.ascontiguousarray(
            np.concatenate([cr[sl].T, sr[sl].T, cc[sl].T, sc[sl].T], axis=0)
        ).astype(np.float32)
        in_maps.append(m)
    return in_maps


def kernel(**inputs):
    nc = _get_built()
    in_maps = _host_prep(inputs)
    res = run_bass_kernel_spmd(nc, in_maps, core_ids=list(range(NC_)))
    logits = np.concatenate(
        [np.asarray(res.results[c]["logits"]).reshape(SL) for c in range(NC_)])
    return logits.astype(np.float32)


if __name__ == "__main__":
    data = np.load("/root/problem/ref_data.npz")
    expected = data["__expected"]
    inputs = {k: data[k] for k in data.files if k != "__expected"}
    got = kernel(**inputs)
    err = np.abs(got - expected)
    rel = np.linalg.norm(got - expected) / np.linalg.norm(expected)
    print("max abs err:", err.max(), "rel l2:", rel)
